# revision 1
# baseline (speedup 1.0000x reference)
"""Trainium2 Bass kernel for a 2-layer masked (ragged) Elman RNN.

Problem: tokens [128,512] -> emb lookup [B,T,1024] -> RNN(1024->2048) ->
RNN(2048->2048) -> final hidden of layer 1, with per-sequence lengths
freezing the hidden state at t >= len (packed-sequence semantics).

Strategy (8 NeuronCores, data-parallel over batch, 16 seqs/core):
  Phase A: embedding gather + bulk input projection xp0 = X@W_ih0 + b0
  Phase B: layer-0 recurrence (512 serial steps), storing transposed
           hidden states y0T per step (doubles as xp1 GEMM input layout)
  Phase C: bulk xp1 = y0 @ W_ih1 + b1 in 64 chunks of 8 timesteps
  Phase D: layer-1 recurrence, storing h1 per step to DRAM; final
           per-sequence capture via indirect gather at row (len-1)*16+b.

The recurrences run unmasked: for t < len the unmasked state equals the
reference's frozen-state values, and the capture row only reads t=len-1.
All matmuls use float32r (TF32-like single-pass fp32, 1 col/cycle at
N>=256; full fp32 is 4x slower).

Matmul layout per step (per core): pre[b,n] = sum_k hT[k,b]^T @ W[k,n]
with the 16-wide batch as the stationary operand (LDWEIGHTS ~ columns,
so cheap) and W streaming at N=512. The tanh output h [16,2048] is
re-transposed on the PE (16 tiles of [16,128]->[128,16]) into the next
step's stationary operand.
"""

import sys

sys.path.insert(0, "/opt/trn_rl_repo")

import numpy as np

B, T, V, D, H = 128, 512, 32000, 1024, 2048
NC = 8
BL = B // NC          # 16 sequences per core
KT = H // 128         # 16 k-tiles of the hidden dim
NT = H // 512         # 4 n-tiles (PSUM bank width)
DKT = D // 128        # 8 k-tiles of the embedding dim
MT = (T * BL) // 128  # 64 token-tiles of 128 rows (t-major)
CH = 128 // BL        # 8 timesteps per xp1 GEMM chunk

STATS = {}
_CACHE = {}


def _build(t_steps):
    import concourse.bass as bass
    import concourse.mybir as mybir
    import concourse.tile as tile
    from concourse import bacc
    from concourse.masks import make_identity

    f32 = mybir.dt.float32
    f32r = mybir.dt.float32r
    i32 = mybir.dt.int32
    Tanh = mybir.ActivationFunctionType.Tanh

    mt = (t_steps * BL) // 128
    nchunk = t_steps // CH

    nc = bacc.Bacc("TRN2", target_bir_lowering=False, debug=False, num_devices=NC)

    tokT = nc.dram_tensor("tokT", [128, mt], i32, kind="ExternalInput")
    cap_idx = nc.dram_tensor("cap_idx", [BL, 1], i32, kind="ExternalInput")
    emb = nc.dram_tensor("emb", [V, D], f32, kind="ExternalInput")
    w_ih0 = nc.dram_tensor("w_ih0", [D, H], f32, kind="ExternalInput")
    w_hh0 = nc.dram_tensor("w_hh0", [H, H], f32, kind="ExternalInput")
    b0 = nc.dram_tensor("b0", [1, H], f32, kind="ExternalInput")
    w_ih1 = nc.dram_tensor("w_ih1", [H, H], f32, kind="ExternalInput")
    w_hh1 = nc.dram_tensor("w_hh1", [H, H], f32, kind="ExternalInput")
    b1 = nc.dram_tensor("b1", [1, H], f32, kind="ExternalInput")
    out_h = nc.dram_tensor("out_h", [BL, H], f32, kind="ExternalOutput")

    xp0_d = nc.dram_tensor("xp0_d", [t_steps * BL, H], f32)
    xp1_d = nc.dram_tensor("xp1_d", [t_steps * BL, H], f32)
    y0T_d = nc.dram_tensor("y0T_d", [t_steps, 128, KT * BL], f32)
    h1_d = nc.dram_tensor("h1_d", [t_steps * BL, H], f32)

    def load_w(W_sb, wsrc, ktiles):
        # W_sb col block (k*NT+n)*512 holds wsrc[k*128:(k+1)*128, n*512:(n+1)*512]
        for k in range(ktiles):
            nc.gpsimd.dma_start(
                W_sb[:, k * H:(k + 1) * H],
                wsrc[k * 128:(k + 1) * 128, :].bitcast(f32r),
            )

    def load_bias(bias_sb, bsrc):
        nc.gpsimd.dma_start(bias_sb[0:1, :], bsrc[0:1, :])
        nc.gpsimd.partition_broadcast(bias_sb[:], bias_sb[0:1, :])

    with tile.TileContext(nc) as tc:
        with (
            tc.tile_pool(name="wpool", bufs=1) as wp,
            tc.tile_pool(name="state", bufs=1) as st,
        ):
            W_sb = wp.tile([128, KT * H], f32r)      # 64KB/partition
            ident = st.tile([128, 128], f32)
            make_identity(nc, ident[:])
            bias_sb = st.tile([128, H], f32)
            zero_sb = st.tile([128, KT * BL], f32)
            nc.gpsimd.memset(zero_sb[:], 0.0)
            tokens_sb = st.tile([128, mt], i32)
            nc.gpsimd.dma_start(tokens_sb[:], tokT[:, :])

            # ---------------- Phase A: embed + xp0 ----------------
            load_w(W_sb, w_ih0, DKT)
            load_bias(bias_sb, b0)
            with (
                nc.named_scope("phaseA"),
                tc.tile_pool(name="ga", bufs=3) as gp,
                tc.tile_pool(name="xt", bufs=2) as xtp,
                tc.tile_pool(name="pa", bufs=2, space="PSUM") as pap,
                tc.tile_pool(name="pn", bufs=4, space="PSUM") as pnp,
                tc.tile_pool(name="ot", bufs=4) as otp,
            ):
                for j in range(mt):
                    xg = gp.tile([128, D], f32)
                    nc.gpsimd.indirect_dma_start(
                        out=xg[:], out_offset=None,
                        in_=emb[:],
                        in_offset=bass.IndirectOffsetOnAxis(
                            ap=tokens_sb[:, j:j + 1], axis=0),
                    )
                    xt_ps = pap.tile([128, D], f32, space="PSUM")
                    for k in range(DKT):
                        nc.tensor.transpose(
                            xt_ps[:, k * 128:(k + 1) * 128],
                            xg[:, k * 128:(k + 1) * 128],
                            ident[:],
                        )
                    xt = xtp.tile([128, D], f32r)
                    nc.vector.tensor_copy(xt[:], xt_ps[:])
                    for n in range(NT):
                        ps = pnp.tile([128, 512], f32, space="PSUM")
                        for k in range(DKT):
                            nc.tensor.matmul(
                                ps[:],
                                lhsT=xt[:, k * 128:(k + 1) * 128],
                                rhs=W_sb[:, (k * NT + n) * 512:(k * NT + n + 1) * 512],
                                start=(k == 0), stop=(k == DKT - 1),
                            )
                        ot = otp.tile([128, 512], f32)
                        nc.vector.tensor_add(
                            ot[:], ps[:], bias_sb[:, n * 512:(n + 1) * 512])
                        nc.gpsimd.dma_start(
                            xp0_d[j * 128:(j + 1) * 128, n * 512:(n + 1) * 512], ot[:])

            # ---------------- recurrence phase builder ----------------
            def recurrence(layer, xp_src):
                with (
                    nc.named_scope(f"rec{layer}"),
                    tc.tile_pool(name=f"st{layer}", bufs=2) as stp,
                    tc.tile_pool(name=f"xp{layer}", bufs=4) as xpp,
                    tc.tile_pool(name=f"hb{layer}", bufs=2) as hbp,
                    tc.tile_pool(name=f"pr{layer}", bufs=6, space="PSUM") as prp,
                    tc.tile_pool(name=f"pt{layer}", bufs=2, space="PSUM") as ptp,
                ):
                    hT_sb = stp.tile([128, KT * BL], f32r, tag="hT")
                    nc.vector.tensor_copy(hT_sb[:], zero_sb[:])
                    for t in range(t_steps):
                        xp_t = xpp.tile([BL, H], f32)
                        nc.gpsimd.dma_start(
                            xp_t[:], xp_src[t * BL:(t + 1) * BL, :])
                        h_bm = hbp.tile([BL, H], f32)
                        tb_ps = ptp.tile([128, NT * 128], f32, space="PSUM")
                        hT_next = stp.tile([128, KT * BL], f32r, tag="hT")
                        hstack = hbp.tile([128, NT * 128], f32, tag="hstack")
                        for n in range(NT):
                            ps = prp.tile([BL, 512], f32, space="PSUM")
                            for k in range(KT):
                                nc.tensor.matmul(
                                    ps[:],
                                    lhsT=hT_sb[:, k * BL:(k + 1) * BL],
                                    rhs=W_sb[:, (k * NT + n) * 512:(k * NT + n + 1) * 512],
                                    start=(k == 0), stop=(k == KT - 1),
                                )
                            nc.vector.tensor_add(
                                ps[:], ps[:], xp_t[:, n * 512:(n + 1) * 512])
                            nc.scalar.activation(
                                h_bm[:, n * 512:(n + 1) * 512], ps[:], Tanh)
                            # stack the 4 k-slices at 32-aligned partition
                            # bases, then ONE wide [128,128] PE transpose per
                            # n-tile instead of four narrow ones (PE transposes
                            # are latency-bound at ~230ns regardless of size)
                            for j in range(4):
                                kk = n * 4 + j
                                nc.vector.tensor_copy(
                                    hstack[32 * j:32 * j + BL,
                                           n * 128:(n + 1) * 128],
                                    h_bm[:, kk * 128:(kk + 1) * 128])
                            nc.tensor.transpose(
                                tb_ps[:, n * 128:(n + 1) * 128],
                                hstack[:, n * 128:(n + 1) * 128],
                                ident[:],
                            )
                            # gather valid columns {32j..32j+16} into the
                            # *other* state buffer (cross-step pipelining)
                            nc.vector.tensor_copy(
                                hT_next[:, n * 64:(n + 1) * 64]
                                .rearrange("p (j c) -> p j c", j=4),
                                tb_ps[:, n * 128:(n + 1) * 128]
                                .rearrange("p (j c) -> p j c", c=32)[:, :, 0:BL])
                        if layer == 0:
                            nc.gpsimd.dma_start(
                                y0T_d[t, :, :], hT_next[:].bitcast(f32))
                        else:
                            nc.gpsimd.dma_start(
                                h1_d[t * BL:(t + 1) * BL, :], h_bm[:])
                        hT_sb = hT_next

            # ---------------- Phase B: layer-0 recurrence ----------------
            load_w(W_sb, w_hh0, KT)
            recurrence(0, xp0_d)

            # ---------------- Phase C: xp1 chunks ----------------
            load_w(W_sb, w_ih1, KT)
            load_bias(bias_sb, b1)
            with (
                nc.named_scope("phaseC"),
                tc.tile_pool(name="lh", bufs=2) as lhp,
                tc.tile_pool(name="pc", bufs=4, space="PSUM") as pcp,
                tc.tile_pool(name="oc", bufs=4) as ocp,
            ):
                for c in range(nchunk):
                    lh = lhp.tile([128, H], f32r)
                    for k in range(KT):
                        nc.gpsimd.dma_start(
                            lh[:, k * 128:(k + 1) * 128]
                            .rearrange("p (t c) -> p t c", t=CH),
                            y0T_d[c * CH:(c + 1) * CH, :, k * BL:(k + 1) * BL]
                            .rearrange("t p c -> p t c").bitcast(f32r),
                        )
                    for n in range(NT):
                        ps = pcp.tile([128, 512], f32, space="PSUM")
                        for k in range(KT):
                            nc.tensor.matmul(
                                ps[:],
                                lhsT=lh[:, k * 128:(k + 1) * 128],
                                rhs=W_sb[:, (k * NT + n) * 512:(k * NT + n + 1) * 512],
                                start=(k == 0), stop=(k == KT - 1),
                            )
                        oc = ocp.tile([128, 512], f32)
                        nc.vector.tensor_add(
                            oc[:], ps[:], bias_sb[:, n * 512:(n + 1) * 512])
                        nc.gpsimd.dma_start(
                            xp1_d[c * 128:(c + 1) * 128, n * 512:(n + 1) * 512],
                            oc[:])

            # ---------------- Phase D: layer-1 recurrence ----------------
            load_w(W_sb, w_hh1, KT)
            recurrence(1, xp1_d)

            # final capture: out[b] = h1 at t = len_b - 1
            with tc.tile_pool(name="cap", bufs=1) as cp:
                ci = cp.tile([BL, 1], i32)
                nc.gpsimd.dma_start(ci[:], cap_idx[:, :])
                og = cp.tile([BL, H], f32)
                nc.gpsimd.indirect_dma_start(
                    out=og[:], out_offset=None,
                    in_=h1_d[:],
                    in_offset=bass.IndirectOffsetOnAxis(ap=ci[:, :1], axis=0),
                )
                nc.gpsimd.dma_start(out_h[:, :], og[:])

    nc.finalize()
    return nc


def _install_ntff_hook():
    """The trimmed agent image lacks antenv.axon_hooks — provide the tiny
    get/set registry and install the ctypes NTFF hook so trace=True works."""
    import types

    if "antenv.axon_hooks" in sys.modules:
        return
    m = types.ModuleType("antenv.axon_hooks")
    _hook = [None]
    m.set_axon_ntff_profile_hook = lambda h: _hook.__setitem__(0, h)
    m.get_axon_ntff_profile_hook = lambda: _hook[0]
    sys.modules["antenv.axon_hooks"] = m
    import antenv
    antenv.axon_hooks = m
    try:
        from trn_agent_boot.trn_boot import _ntff_profile_via_ctypes
        hook = _ntff_profile_via_ctypes("/opt/axon/libaxon_pjrt.so")
        if hook is not None:
            m.set_axon_ntff_profile_hook(hook)
        import concourse.bass_utils as bu
        bu.upload_artifacts = lambda d: str(d)
    except Exception:
        pass


def kernel(tokens, lengths, emb, W_ih0, W_hh0, b0, W_ih1, W_hh1, b1,
           _t_steps=T, _trace=False):
    from concourse.bass_utils import run_bass_kernel_spmd

    if _trace:
        _install_ntff_hook()

    tokens = np.asarray(tokens).astype(np.int32)
    lengths = np.asarray(lengths).astype(np.int32)
    emb = np.ascontiguousarray(np.asarray(emb, dtype=np.float32))
    W_ih0 = np.ascontiguousarray(np.asarray(W_ih0, dtype=np.float32))
    W_hh0 = np.ascontiguousarray(np.asarray(W_hh0, dtype=np.float32))
    W_ih1 = np.ascontiguousarray(np.asarray(W_ih1, dtype=np.float32))
    W_hh1 = np.ascontiguousarray(np.asarray(W_hh1, dtype=np.float32))
    b0 = np.ascontiguousarray(np.asarray(b0, dtype=np.float32).reshape(1, H))
    b1 = np.ascontiguousarray(np.asarray(b1, dtype=np.float32).reshape(1, H))

    ts = _t_steps
    if ts not in _CACHE:
        _CACHE[ts] = _build(ts)
    nc = _CACHE[ts]

    in_maps = []
    for c in range(NC):
        tok_c = tokens[c * BL:(c + 1) * BL, :ts]          # [16, ts]
        flat = tok_c.T.reshape(-1)                        # t-major rows
        tokT = np.ascontiguousarray(flat.reshape(-1, 128).T)  # [128, mt]
        len_c = np.minimum(lengths[c * BL:(c + 1) * BL].astype(np.int64), ts)
        cap = ((len_c - 1) * BL + np.arange(BL)).astype(np.int32)[:, None]
        in_maps.append({
            "tokT": tokT,
            "cap_idx": np.ascontiguousarray(cap),
            "emb": emb,
            "w_ih0": W_ih0, "w_hh0": W_hh0, "b0": b0,
            "w_ih1": W_ih1, "w_hh1": W_hh1, "b1": b1,
        })

    res = run_bass_kernel_spmd(nc, in_maps, list(range(NC)), trace=_trace)
    STATS["exec_time_ns"] = res.exec_time_ns
    STATS["mean_exec_time_ns"] = res.mean_exec_time_ns
    STATS["scope_times"] = res.per_core_scope_times
    out = np.concatenate([res.results[c]["out_h"] for c in range(NC)], axis=0)
    return out.astype(np.float32)

